# revision 8
# baseline (speedup 1.0000x reference)
"""Entmax attention Trainium2 kernel (8-core SPMD, head-parallel).

Math (matches the reference _entmax_naive exactly):
  q,k,v projections (fp32)  ->  scores = (q*scale) @ k^T  (fp32, causal)
  per row: k_support = #{j : s_j > tau*} where tau* solves sum relu(s - tau*) = 1
           (solved by Newton iteration: 9 iters on bf16 scores + 2 fp32 polish,
            validated exact vs the sort-based reference on all 32768 rows)
  tau_star = (row_sum - 1)/k_support ; p = relu(s - tau_star) ; attn = p/sum(p)
  out = attn @ v ; final = out @ Wo^T  (per-core partial, summed on host)

Sharding: 16 heads / 8 cores = 2 heads per core. Each core computes its heads'
attention output [2048,128] and the partial Wo product [2048,1024]; host sums
the 8 partials.
"""
import os
import numpy as np
from contextlib import ExitStack

import concourse.bass as bass
import concourse.tile as tile
import concourse.mybir as mybir
from concourse import bacc
from concourse.bass_utils import run_bass_kernel_spmd

L = 2048
D = 1024
H = 16
HD = 64
N_CORES = 8
HPC = 2  # heads per core
SCALE = float(HD) ** -0.5

FP32 = mybir.dt.float32
BF16 = mybir.dt.bfloat16
Alu = mybir.AluOpType
Act = mybir.ActivationFunctionType

N_BF16_ITERS = 9
N_F32_ITERS = 2
NEG_BIG = -1.0e30
MAX_INIT = -3.0e38
# F-pass engine split thresholds (row-block index; n = 128*(rb+1))
ACT_F_MIN_RB_BF16 = 10   # bf16 iters: rb >= this -> F on ACT, else DVE
ACT_F_MIN_RB_F32 = 5     # f32 polish iters

# pairs (a, b) with (a+1)+(b+1) = 17 -> constant pair width 2176
RB_PAIRS = [(0, 15), (4, 11), (1, 14), (5, 10), (2, 13), (6, 9), (3, 12), (7, 8)]
PAIR_W = 17 * 128  # 2176


def _units_of_group(g):
    """8 units per group: 2 rb-pairs x {A,B} x 2 heads.

    Returns list of (rb, head, pair_slot, col_off) where pair_slot in 0..3
    identifies the S tile (pair x head) and col_off is the unit's column
    offset inside that pair tile.
    """
    units = []
    for pi_local, pair in enumerate((RB_PAIRS[2 * g], RB_PAIRS[2 * g + 1])):
        ra, rb_ = pair
        na = 128 * (ra + 1)
        for h in range(HPC):
            slot = pi_local * 2 + h
            units.append((ra, h, slot, 0))
            units.append((rb_, h, slot, na))
    return units


def build_program(n_groups=4, do_newton=True, do_avwo=True, debug_out=None,
                  sc_level=99):
    # debug_out: None | "qkv" | "scores"  -- early-exit paths that DMA
    # intermediates into `out` for hardware bisection.
    nc = bacc.Bacc("TRN2", target_bir_lowering=False, debug=False, num_devices=1)

    xT_d = nc.dram_tensor("xT", [D, L], FP32, kind="ExternalInput")
    wq_d = nc.dram_tensor("wqT", [D, 128], FP32, kind="ExternalInput")
    wk_d = nc.dram_tensor("wkT", [D, 128], FP32, kind="ExternalInput")
    wv_d = nc.dram_tensor("wvT", [D, 128], FP32, kind="ExternalInput")
    wo_d = nc.dram_tensor("woT", [128, D], FP32, kind="ExternalInput")
    mneg_d = nc.dram_tensor("mneg", [128, 128], FP32, kind="ExternalInput")
    m01_d = nc.dram_tensor("m01", [128, 128], FP32, kind="ExternalInput")
    ident_d = nc.dram_tensor("ident", [128, 128], FP32, kind="ExternalInput")
    out_d = nc.dram_tensor("out", [L, D], FP32, kind="ExternalOutput")

    with tile.TileContext(nc) as tc:
        with ExitStack() as ctx:
            # ---------- persistent pools ----------
            persist = ctx.enter_context(tc.tile_pool(name="persist", bufs=1))
            qT = persist.tile([128, L], FP32, tag="qT")       # [d(2 heads), i]
            kT = persist.tile([128, L], FP32, tag="kT")       # [d(2 heads), j]
            vt = persist.tile([128, 16, 128], FP32, tag="vt")  # [j in tile, jt, d]
            woT = persist.tile([128, D], FP32, tag="woT")      # [d, o]
            mneg = persist.tile([128, 128], FP32, tag="mneg")
            m01 = persist.tile([128, 128], FP32, tag="m01")
            ident = persist.tile([128, 128], FP32, tag="ident")
            zeros_bf = persist.tile([128, L], BF16, tag="zbf")
            zeros_f = persist.tile([128, L], FP32, tag="zf")
            trash_a = persist.tile([128, L], BF16, tag="tra")   # ACT F-pass sink
            trash_d = persist.tile([128, L], BF16, tag="trd")   # DVE F-pass sink
            trash_c = persist.tile([128, L], BF16, tag="trc")   # DVE cnt-pass sink

            # per-unit stats, column = group*8 + unit_idx
            NST = 32

            def stat(tag):
                return persist.tile([128, NST], FP32, tag=tag, name=tag)

            maxF, maxD = stat("maxF"), stat("maxD")
            sumF, sumD = stat("sumF"), stat("sumD")
            mx, sm = stat("mx"), stat("sm")
            Tt, nT = stat("T"), stat("nT")
            Ft, Ct = stat("F"), stat("C")
            rec, Fm, dlt = stat("rec"), stat("Fm"), stat("dlt")
            tau, ntau = stat("tau"), stat("ntau")
            sump, rz = stat("sump"), stat("rz")

            nc.sync.dma_start(mneg[:], mneg_d.ap())
            nc.sync.dma_start(m01[:], m01_d.ap())
            nc.sync.dma_start(ident[:], ident_d.ap())
            nc.sync.dma_start(woT[:], wo_d.ap())
            nc.vector.memset(zeros_bf[:], 0.0)
            nc.vector.memset(zeros_f[:], 0.0)
            nc.vector.memset(maxF[:], MAX_INIT)
            nc.vector.memset(sumF[:], 0.0)
            nc.vector.memset(maxD[:], MAX_INIT)
            nc.vector.memset(sumD[:], 0.0)

            # ---------- phase 1: projections ----------
            with ExitStack() as p1:
                ph1 = p1.enter_context(tc.tile_pool(name="ph1", bufs=1))
                ph1p = p1.enter_context(
                    tc.tile_pool(name="ph1p", bufs=2, space="PSUM"))
                xt = ph1.tile([128, 8, L], FP32, tag="xt")
                wqs = ph1.tile([128, 8, 128], FP32, tag="wqs")
                wks = ph1.tile([128, 8, 128], FP32, tag="wks")
                wvs = ph1.tile([128, 8, 128], FP32, tag="wvs")

                xview = xT_d.ap().rearrange("(c p) n -> p c n", p=128)
                for c in range(8):
                    nc.sync.dma_start(xt[:, c, :], xview[:, c, :])
                nc.sync.dma_start(wqs[:], wq_d.ap().rearrange("(c p) m -> p c m", p=128))
                nc.sync.dma_start(wks[:], wk_d.ap().rearrange("(c p) m -> p c m", p=128))
                nc.sync.dma_start(wvs[:], wv_d.ap().rearrange("(c p) m -> p c m", p=128))

                for dst, wsb in ((qT, wqs), (kT, wks)):
                    for ic in range(4):
                        ps = ph1p.tile([128, 512], FP32, tag="pp")
                        for e in range(8):
                            nc.tensor.matmul(
                                ps[:], wsb[:, e, :], xt[:, e, 512 * ic:512 * (ic + 1)],
                                start=(e == 0), stop=(e == 7))
                        if ic % 2 == 0:
                            nc.scalar.copy(dst[:, 512 * ic:512 * (ic + 1)], ps[:])
                        else:
                            nc.vector.tensor_copy(dst[:, 512 * ic:512 * (ic + 1)], ps[:])
                for jt in range(16):
                    ps = ph1p.tile([128, 512], FP32, tag="pp")
                    for e in range(8):
                        nc.tensor.matmul(
                            ps[:, :128], xt[:, e, 128 * jt:128 * (jt + 1)], wvs[:, e, :],
                            start=(e == 0), stop=(e == 7))
                    if jt % 2 == 0:
                        nc.scalar.copy(vt[:, jt, :], ps[:, :128])
                    else:
                        nc.vector.tensor_copy(vt[:, jt, :], ps[:, :128])

            if debug_out == "qkv":
                flat = out_d.ap().rearrange("a b -> (a b)")
                nc.sync.dma_start(flat[0:262144], qT[:])
                nc.sync.dma_start(flat[262144:524288], kT[:])
                nc.sync.dma_start(flat[524288:786432],
                                  vt[:].rearrange("p a b -> p (a b)"))

            # ---------- phase 2 pools ----------
            s_pool = ctx.enter_context(tc.tile_pool(name="spair", bufs=2))
            sb_pool = ctx.enter_context(tc.tile_pool(name="sbpair", bufs=2))
            ps_sc = ctx.enter_context(tc.tile_pool(name="ps_sc", bufs=1, space="PSUM"))
            ps_av = ctx.enter_context(tc.tile_pool(name="ps_av", bufs=2, space="PSUM"))
            ps_tr = ctx.enter_context(tc.tile_pool(name="ps_tr", bufs=2, space="PSUM"))
            ptb_pool = ctx.enter_context(tc.tile_pool(name="ptb", bufs=2))
            oc_pool = ctx.enter_context(tc.tile_pool(name="oc", bufs=2))
            wo_pool = ctx.enter_context(tc.tile_pool(name="woout", bufs=2))

            copy_flip = [0]

            def balanced_copy(dst, src):
                if copy_flip[0] % 2 == 0:
                    nc.scalar.copy(dst, src)
                else:
                    nc.vector.tensor_copy(dst, src)
                copy_flip[0] += 1

            for g in range(n_groups):
                units = _units_of_group(g)
                gsl = slice(8 * g, 8 * g + 8)
                # pair tiles for this group: slot -> (S fp32, S bf16)
                Sg = [s_pool.tile([128, PAIR_W], FP32, tag=f"sp{s}", name=f"sp{s}_{g}") for s in range(4)]
                Sbg = [sb_pool.tile([128, PAIR_W], BF16, tag=f"sb{s}", name=f"sb{s}_{g}") for s in range(4)]

                # ---- A/B: scores -> S (fp32, masked), Sb (bf16), row sums/maxes ----
                for ui, (rb, h, slot, off) in enumerate(units):
                    col = 8 * g + ui
                    n = 128 * (rb + 1)
                    full = n - 128
                    S, Sb = Sg[slot], Sbg[slot]
                    ps = ps_sc.tile([128, 2048], FP32, tag="sc")
                    for c0 in range(0, n, 512):
                        w = min(512, n - c0)
                        nc.tensor.matmul(
                            ps[:, c0:c0 + w],
                            qT[64 * h:64 * h + 64, 128 * rb:128 * rb + 128],
                            kT[64 * h:64 * h + 64, c0:c0 + w],
                            start=True, stop=True)
                    if full > 0:
                        if sc_level >= 1:
                            # copy psum->Sb (bf16) + row-sum of full region (ACT)
                            nc.scalar.activation(
                                Sb[:, off:off + full], ps[:, :full], Act.Identity,
                                bias=0.0, accum_out=sumF[:, col:col + 1])
                        # copy psum->S (fp32) + row-max of full region (DVE)
                        nc.vector.tensor_scalar(
                            out=S[:, off:off + full], in0=ps[:, :full],
                            scalar1=0.0, scalar2=MAX_INIT,
                            op0=Alu.add, op1=Alu.max,
                            accum_out=maxF[:, col:col + 1])
                    # diagonal 128 cols: mask to -1e30 into S, then row-max
                    # (tensor_tensor_reduce compiles but dies on HW, so two ops)
                    nc.vector.tensor_tensor(
                        S[:, off + full:off + n], ps[:, full:n], mneg[:], Alu.add)
                    if sc_level >= 2:
                        nc.vector.tensor_scalar(
                            out=trash_c[:, :128], in0=S[:, off + full:off + n],
                            scalar1=0.0, scalar2=MAX_INIT,
                            op0=Alu.add, op1=Alu.max,
                            accum_out=maxD[:, col:col + 1])
                    if sc_level >= 3:
                        # diag row-sum of valid entries (masked multiplicatively)
                        nc.vector.scalar_tensor_tensor(
                            out=trash_d[:, :128],
                            in0=ps[:, full:n], scalar=1.0, in1=m01[:],
                            op0=Alu.mult, op1=Alu.mult,
                            accum_out=sumD[:, col:col + 1])
                    if sc_level >= 4:
                        # masked diag -> Sb
                        nc.vector.tensor_copy(Sb[:, off + full:off + n], S[:, off + full:off + n])

                # ---- combine stats, init T/negT ----
                nc.vector.tensor_tensor(mx[:, gsl], maxF[:, gsl], maxD[:, gsl], Alu.max)
                nc.vector.tensor_tensor(sm[:, gsl], sumF[:, gsl], sumD[:, gsl], Alu.add)
                nc.vector.tensor_scalar_add(Tt[:, gsl], mx[:, gsl], -1.0)
                nc.vector.tensor_scalar(
                    out=nT[:, gsl], in0=mx[:, gsl], scalar1=-1.0, scalar2=1.0,
                    op0=Alu.mult, op1=Alu.add)

                # ---- Newton iterations ----
                def newton_iter(use_bf16):
                    for ui, (rb, h, slot, off) in enumerate(units):
                        col = 8 * g + ui
                        n = 128 * (rb + 1)
                        Ssrc = Sbg[slot] if use_bf16 else Sg[slot]
                        zt = zeros_bf if use_bf16 else zeros_f
                        thr = ACT_F_MIN_RB_BF16 if use_bf16 else ACT_F_MIN_RB_F32
                        if rb >= thr:
                            nc.scalar.activation(
                                trash_a[:, :n], Ssrc[:, off:off + n], Act.Relu,
                                bias=nT[:, col:col + 1],
                                accum_out=Ft[:, col:col + 1])
                        else:
                            nc.vector.scalar_tensor_tensor(
                                out=trash_d[:, :n], in0=Ssrc[:, off:off + n],
                                scalar=nT[:, col:col + 1], in1=zt[:, :n],
                                op0=Alu.add, op1=Alu.max,
                                accum_out=Ft[:, col:col + 1])
                        nc.vector.tensor_scalar(
                            out=trash_c[:, :n], in0=Ssrc[:, off:off + n],
                            scalar1=Tt[:, col:col + 1], scalar2=0.0,
                            op0=Alu.is_gt, op1=Alu.add,
                            accum_out=Ct[:, col:col + 1])
                    nc.vector.tensor_scalar_max(Ct[:, gsl], Ct[:, gsl], 1.0)
                    nc.vector.reciprocal(rec[:, gsl], Ct[:, gsl])
                    nc.vector.tensor_scalar_add(Fm[:, gsl], Ft[:, gsl], -1.0)
                    nc.vector.tensor_tensor(dlt[:, gsl], Fm[:, gsl], rec[:, gsl], Alu.mult)
                    nc.vector.tensor_tensor(Tt[:, gsl], Tt[:, gsl], dlt[:, gsl], Alu.add)
                    nc.vector.tensor_tensor(nT[:, gsl], nT[:, gsl], dlt[:, gsl], Alu.subtract)

                if debug_out == "scores":
                    flat2 = out_d.ap().rearrange("a b -> (a b)")
                    for slot in range(4):
                        nc.sync.dma_start(
                            flat2[278528 * slot:278528 * (slot + 1)], Sg[slot][:])
                    continue
                if not do_newton:
                    continue
                for _ in range(N_BF16_ITERS):
                    newton_iter(True)
                for _ in range(N_F32_ITERS):
                    newton_iter(False)

                # ---- final count -> k_support; tau_star ----
                for ui, (rb, h, slot, off) in enumerate(units):
                    col = 8 * g + ui
                    n = 128 * (rb + 1)
                    nc.vector.tensor_scalar(
                        out=trash_c[:, :n], in0=Sg[slot][:, off:off + n],
                        scalar1=Tt[:, col:col + 1], scalar2=0.0,
                        op0=Alu.is_gt, op1=Alu.add,
                        accum_out=Ct[:, col:col + 1])
                nc.vector.tensor_scalar_max(Ct[:, gsl], Ct[:, gsl], 1.0)
                nc.vector.reciprocal(rec[:, gsl], Ct[:, gsl])
                nc.vector.tensor_scalar_add(Fm[:, gsl], sm[:, gsl], -1.0)
                nc.vector.tensor_tensor(tau[:, gsl], Fm[:, gsl], rec[:, gsl], Alu.mult)
                nc.vector.tensor_scalar_mul(ntau[:, gsl], tau[:, gsl], -1.0)

                if not do_avwo:
                    continue
                # ---- p = relu(S - tau) in place + row sum; AV; Wo ----
                outc_of_rb = {}
                for ui, (rb, h, slot, off) in enumerate(units):
                    col = 8 * g + ui
                    n = 128 * (rb + 1)
                    S = Sg[slot]
                    nc.scalar.activation(
                        S[:, off:off + n], S[:, off:off + n], Act.Relu,
                        bias=ntau[:, col:col + 1],
                        accum_out=sump[:, col:col + 1])
                    # rz = 1/(sump + 1e-10)
                    nc.vector.tensor_scalar_add(
                        Fm[:, col:col + 1], sump[:, col:col + 1], 1.0e-10)
                    nc.vector.reciprocal(rz[:, col:col + 1], Fm[:, col:col + 1])

                    if rb not in outc_of_rb:
                        outc_of_rb[rb] = oc_pool.tile([128, 128], FP32, tag=f"oc{ui % 2}", name=f"oc{g}_{rb}")
                    outc = outc_of_rb[rb]

                    av = ps_av.tile([128, 512], FP32, tag="av")
                    nt = n // 128
                    for c0 in range(0, nt, 4):
                        cw = min(4, nt - c0)
                        pt_ps = ps_tr.tile([128, 512], FP32, tag="tr")
                        for c in range(cw):
                            jt = c0 + c
                            nc.tensor.transpose(
                                pt_ps[:, 128 * c:128 * (c + 1)],
                                S[:, off + 128 * jt:off + 128 * (jt + 1)], ident[:])
                        pt_sb = ptb_pool.tile([128, 512], FP32, tag="ptb")
                        balanced_copy(pt_sb[:, :128 * cw], pt_ps[:, :128 * cw])
                        for c in range(cw):
                            jt = c0 + c
                            nc.tensor.matmul(
                                av[:, :64], pt_sb[:, 128 * c:128 * (c + 1)],
                                vt[:, jt, 64 * h:64 * h + 64],
                                start=(jt == 0), stop=(jt == nt - 1))
                    # normalize while copying out of psum
                    nc.scalar.activation(
                        outc[:, 64 * h:64 * h + 64], av[:, :64], Act.Copy,
                        bias=0.0, scale=rz[:, col:col + 1])

                    if h == 1:
                        # both heads done -> Wo partial for this rb
                        ot_ps = ps_tr.tile([128, 512], FP32, tag="tr")
                        nc.tensor.transpose(ot_ps[:, :128], outc[:], ident[:])
                        otb = ptb_pool.tile([128, 512], FP32, tag="ptb")
                        balanced_copy(otb[:, :128], ot_ps[:, :128])
                        wo_out = wo_pool.tile([128, D], FP32, tag="wod")
                        for oc2 in range(2):
                            wps = ps_av.tile([128, 512], FP32, tag="av")
                            nc.tensor.matmul(
                                wps[:], otb[:, :128], woT[:, 512 * oc2:512 * (oc2 + 1)],
                                start=True, stop=True)
                            balanced_copy(wo_out[:, 512 * oc2:512 * (oc2 + 1)], wps[:])
                        nc.sync.dma_start(
                            out_d.ap()[128 * rb:128 * (rb + 1), :], wo_out[:])

    nc.compile()
    return nc


_CACHE = {}


def _get_nc():
    if "nc" not in _CACHE:
        _CACHE["nc"] = build_program()
    return _CACHE["nc"]


def _host_inputs(x, Wq, Wk, Wv, Wo):
    xT = np.ascontiguousarray(x[0].T).astype(np.float32)
    ii = np.arange(128)
    mneg = np.where(ii[None, :] > ii[:, None], np.float32(NEG_BIG), np.float32(0.0)).astype(np.float32)
    m01 = (ii[None, :] <= ii[:, None]).astype(np.float32)
    ident = np.eye(128, dtype=np.float32)
    in_maps = []
    for c in range(N_CORES):
        hsl = slice(128 * c, 128 * (c + 1))
        in_maps.append({
            "xT": xT,
            "wqT": np.ascontiguousarray((Wq[hsl] * np.float32(SCALE)).T).astype(np.float32),
            "wkT": np.ascontiguousarray(Wk[hsl].T).astype(np.float32),
            "wvT": np.ascontiguousarray(Wv[hsl].T).astype(np.float32),
            "woT": np.ascontiguousarray(Wo[:, hsl].T).astype(np.float32),
            "mneg": mneg,
            "m01": m01,
            "ident": ident,
        })
    return in_maps


def kernel(x, Wq, Wk, Wv, Wo, _trace=False):
    nc = _get_nc()
    in_maps = _host_inputs(np.asarray(x), np.asarray(Wq), np.asarray(Wk),
                           np.asarray(Wv), np.asarray(Wo))
    res = run_bass_kernel_spmd(nc, in_maps, core_ids=list(range(N_CORES)),
                               trace=_trace)
    out = np.zeros((L, D), np.float32)
    for c in range(N_CORES):
        out += res.results[c]["out"]
    if _trace:
        _CACHE["last_results"] = res
    return out.reshape(1, L, D)
